# revision 11
# baseline (speedup 1.0000x reference)
"""TRN2 Bass kernel for nn_AVNNType1Linear.

Reference computation (B=2048, D_in=D_out=4096):
    act, carry = x[..., 0], x[..., 1]                  # x: [B, D_in, 2] f32
    act_out    = relu(act @ W.T + b)                   # [B, D_out]
    val        = 0.5*(mean(act, -1) + mean(carry, -1)) # [B]
    out        = stack([act_out, broadcast(val)], -1)  # [B, D_out, 2]

Distribution over 8 NeuronCores: 4-way data-parallel over batch x 2-way
tensor-parallel over output features. Per-core f16 GEMM [512b x 4096k x
2048o] = 512 matmuls of [128k x 128b] x [128k x 512o], fp32 PSUM.

The kernel is PE-issue-rate-bound: 512 MMs x ~216ns = 110.6us floor.
Key measured pitfalls this version engineers around:
  * Single-queue DMA: with every stream on the Sync HWDGE ring, w-tile
    delivery paced the matmuls to 259ns. The w stream owns the Sync ring
    exclusively; act/s/bias loads ride the Scalar (Activation) HWDGE
    ring; stores ride GpSimd SWDGE + the Scalar ring.
  * DMAHW semaphore-lane aliasing: a late s-row load (WAR-blocked behind
    the serial Scalar-engine reduction chain at sn bufs=2) shared a
    round-robin completion lane with a tail w tile and stalled the PE
    2.6us mid-kernel (then a HAM cold restart). s loads now issue early
    from a bufs=4 pool so every load completes long before the tail.
  * Startup: the PE-warmup tile is memset by the (idle) Vector engine
    instead of DMA'd, so warmup matmuls start right at preamble end and
    genuinely overlap the first w/act DMA wait (HAM at 8/8 when real
    data lands).

Traffic (~29MB/core vs 36MB before): the derive's operand arrives as a
host-prepped s = act+carry natural-layout f16 tensor (4MB) whose row
sums (-> val, the carry channel's broadcast mean) accumulate on the
otherwise-idle Scalar engine; bias leaves the PE (no ones-row K=1
matmuls) and is added in the DVE epilogue from a host-pre-broadcast
[128, O_LOC] f32 tile; the kernel stores only the act channel
([B_LOC, O_LOC] f32, 4KB DMA lines via per-(j,o-pair) [128, 1024]
tiles) plus the [B_LOC] val vector. The host assembles the interleaved
[B, D, 2] output and broadcasts val into the carry channel during
gather/unshard.

Each o-pair k-sweep keeps all 8 PSUM banks live (stationary act tile
feeds 2 consecutive matmuls) and ends with a j-major staggered tail so
every batch tile's epilogue + store streams out while later batch tiles
still compute. The very last batch tile runs its tail ot-major so the
final epilogue + store cover only [128, 512] on the fast Scalar ring.
"""

import os

import numpy as np

import concourse.mybir as mybir
import concourse.tile as tile
from concourse import bacc
from concourse.bass_utils import run_bass_kernel_spmd

B, D = 2048, 4096          # batch, D_in == D_out
M_SHARDS, F_SHARDS = 4, 2  # batch x feature grid over 8 cores
B_LOC = B // M_SHARDS      # 512 batch rows per core
O_LOC = D // F_SHARDS      # 2048 output features per core
KT = D // 128              # 32 contraction tiles
OT = O_LOC // 512          # 4 output tiles of 512
JT = B_LOC // 128          # 4 batch tiles of 128
KG = 8                     # activator SBUF tiles (groups of KT//KG k-tiles)
KPG = KT // KG

MM_DTYPE = os.environ.get("MM_DTYPE", "float16")


def _build():
    dt = mybir.dt
    mmdt = getattr(dt, MM_DTYPE)
    nc = bacc.Bacc("TRN2", target_bir_lowering=False, debug=False)
    xg = nc.dram_tensor("xg", [KG * 128, KPG * B_LOC], mmdt, kind="ExternalInput").ap()
    sN = nc.dram_tensor("sN", [B_LOC, D], mmdt, kind="ExternalInput").ap()
    wQ = nc.dram_tensor(
        "wQ", [(OT // 2) * (KT // 2) * 128, 2 * 2 * 512], mmdt, kind="ExternalInput"
    ).ap()
    biasb = nc.dram_tensor("biasb", [128, O_LOC], dt.float32, kind="ExternalInput").ap()
    biasr = nc.dram_tensor("biasr", [1, O_LOC], mmdt, kind="ExternalInput").ap()
    out = nc.dram_tensor("out", [B_LOC, O_LOC], dt.float32, kind="ExternalOutput").ap()
    val_out = nc.dram_tensor("val", [128, JT], dt.float32, kind="ExternalOutput").ap()

    with tile.TileContext(nc) as tc:
        with (
            tc.tile_pool(name="persist", bufs=1) as persist,
            tc.tile_pool(name="wstream", bufs=8) as wpool,
            tc.tile_pool(name="snpool", bufs=4) as snpool,
            tc.tile_pool(name="opool", bufs=4) as opool,
            tc.tile_pool(name="small", bufs=1) as small,
            tc.tile_pool(name="ps", bufs=8, space="PSUM") as pspool,
        ):
            # PE-warmup stationary/moving tile: engine-memset (no DMA dep)
            # so warmup matmuls start the instant the preamble ends
            ones_sb = persist.tile([128, 128], mmdt)
            nc.vector.memset(ones_sb, 1.0)
            bias_sb = persist.tile([128, O_LOC], dt.float32)

            # whole activator shard, [i%128, kt, b] layout, in KG chunks,
            # all on the Scalar HWDGE ring (keeps the Sync ring w-only).
            act_g = [
                persist.tile([128, KPG, B_LOC], mmdt, tag=f"act{g}", name=f"act{g}")
                for g in range(KG)
            ]

            def load_act(g):
                nc.scalar.dma_start(
                    out=act_g[g],
                    in_=xg[g * 128 : (g + 1) * 128, :].rearrange(
                        "p (kt b) -> p kt b", kt=KPG
                    ),
                )

            load_act(0)

            def act_tile(k, j):
                return act_g[k // KPG][:, k % KPG, j * 128 : (j + 1) * 128]

            # per-row sums of s = act+carry -> val; loads on the Scalar
            # ring, reduction on the Scalar engine (Copy accum_out). The
            # activation's main output is a don't-care: all four share one
            # scratch tile (the Scalar engine serializes them anyway).
            csum_sb = small.tile([128, JT], dt.float32)
            val_sb = small.tile([128, JT], dt.float32)
            scr_sb = small.tile([128, D], mmdt)

            def load_s(j):
                s_t = snpool.tile([128, D], mmdt, tag="sn", name=f"sn_{j}")
                nc.scalar.dma_start(out=s_t, in_=sN[j * 128 : (j + 1) * 128, :])
                return s_t

            def row_sums(s_t, j):
                nc.scalar.activation(
                    scr_sb, s_t, mybir.ActivationFunctionType.Copy,
                    accum_out=csum_sb[:, j : j + 1],
                )

            # o-pairs: all 8 PSUM banks live so each stationary act tile
            # feeds 2 consecutive matmuls (the weight-load cost amortizes)
            STAG = 12  # k-chunks before the j-major staggered tail

            def epilogue_half(ps_t, j, o, eng, with_bias=True):
                # one o-tile; 2KB-line store. with_bias=False means bias was
                # already matmul-accumulated into PSUM: single ReLU pass.
                o_sl = slice(o * 512, (o + 1) * 512)
                out_t = opool.tile(
                    [128, 512], dt.float32, tag="outh", name=f"outh_{o}_{j}"
                )
                if with_bias:
                    nc.vector.tensor_tensor(
                        out_t, ps_t, bias_sb[:, o_sl], op=mybir.AluOpType.add
                    )
                    nc.vector.tensor_scalar_max(out_t, out_t, 0.0)
                else:
                    nc.vector.tensor_scalar_max(out_t, ps_t, 0.0)
                eng.dma_start(out=out[j * 128 : (j + 1) * 128, o_sl], in_=out_t)

            def epilogue_pair(ps_pair, j, op_, eng):
                # relu(psum + bias) for both o-tiles of the pair into one
                # [128, 1024] f32 tile -> single 4KB-line store
                pair_sl = slice(2 * op_ * 512, (2 * op_ + 2) * 512)
                out_t = opool.tile(
                    [128, 2, 512], dt.float32, tag="out", name=f"out_{op_}_{j}"
                )
                for ot in range(2):
                    o_sl = slice((2 * op_ + ot) * 512, (2 * op_ + ot + 1) * 512)
                    nc.vector.tensor_tensor(
                        out_t[:, ot, :], ps_pair[ot], bias_sb[:, o_sl],
                        op=mybir.AluOpType.add,
                    )
                    nc.vector.tensor_scalar_max(out_t[:, ot, :], out_t[:, ot, :], 0.0)
                eng.dma_start(
                    out=out[j * 128 : (j + 1) * 128, pair_sl].rearrange(
                        "p (ot n) -> p ot n", n=512
                    ),
                    in_=out_t,
                )

            s_tiles = {}
            for op_ in range(OT // 2):
                o0 = 2 * op_
                pair_sl = slice(o0 * 512, (o0 + 2) * 512)
                last = op_ == OT // 2 - 1
                ps = [
                    [
                        pspool.tile(
                            [128, 512], dt.float32, tag="ps", name=f"ps_{o0}_{ot}_{j}"
                        )
                        for j in range(JT)
                    ]
                    for ot in range(2)
                ]
                if op_ == 0:
                    # PE warmup during the startup DMA wait: matmuls on the
                    # memset tile flip the HAM clock gate to 8/8 before the
                    # real data lands; the garbage lands in ps[0][0] /
                    # ps[1][0], which the first real start=True matmuls
                    # clear anyway.
                    for i in range(40):
                        nc.tensor.matmul(
                            ps[i % 2][0][:, 0:128],
                            ones_sb,
                            ones_sb,
                            start=True, stop=True,
                            skip_group_check=True,
                        )
                w_tiles = {}
                for kp in range(KT // 2):
                    if op_ == 0 and 1 <= kp <= 4:
                        load_act(2 * kp - 1)
                        if 2 * kp <= KG - 1:
                            load_act(2 * kp)
                    if op_ == 0 and kp == 6:
                        nc.scalar.dma_start(out=bias_sb, in_=biasb)
                    # the val chain (s loads -> Scalar row sums -> val store
                    # -> biasr row) runs in pair-1's DMA-quiet main loop so
                    # none of its completions can alias a DMAHW lane with a
                    # staggered-tail w tile (the measured bimodal +6us stall)
                    if op_ == 1 and 0 <= kp <= 3:
                        s_tiles[kp] = load_s(kp)
                    if op_ == 1 and 4 <= kp <= 7:
                        row_sums(s_tiles[kp - 4], kp - 4)
                    if op_ == 1 and kp == 8:
                        nc.scalar.dma_start(out=scr_sb[0:1, 0:O_LOC], in_=biasr)
                    w_t = wpool.tile(
                        [128, 2, 2, 512], mmdt, tag="wt", name=f"wt_{o0}_{kp}"
                    )
                    nc.sync.dma_start(
                        out=w_t,
                        in_=wQ[
                            (op_ * (KT // 2) + kp) * 128 : (op_ * (KT // 2) + kp + 1)
                            * 128,
                            :,
                        ].rearrange("p (kt ot n) -> p kt ot n", kt=2, ot=2),
                    )
                    w_tiles[kp] = w_t
                    if kp >= STAG:
                        continue  # tail k-chunks run j-major below
                    # two k-tiles per w fetch, two o-tiles per stationary
                    for kk in range(2):
                        k = 2 * kp + kk
                        for j in range(JT):
                            for ot in range(2):
                                nc.tensor.matmul(
                                    ps[ot][j], act_tile(k, j), w_t[:, kk, ot, :],
                                    start=(k == 0), stop=(k == KT - 1),
                                )
                if op_ == 1:
                    # val = total row sum / (2*D); tiny result, GpSimd queue
                    nc.vector.tensor_scalar_mul(val_sb, csum_sb, 1.0 / (2 * D))
                    nc.gpsimd.dma_start(out=val_out, in_=val_sb)
                # staggered j-major tail: each batch tile finishes its
                # remaining k-chunks and streams its epilogue + store out
                # while the later batch tiles still compute. The final
                # batch tile of the final pair runs ot-major so the last
                # epilogue + store cover only one [128, 512] o-tile.
                for j in range(JT):
                    final = last and j == JT - 1
                    if final:
                        for ot in range(2):
                            o = o0 + ot
                            for kp in range(STAG, KT // 2):
                                for kk in range(2):
                                    k = 2 * kp + kk
                                    if k == KT - 1:
                                        # bias rides the accumulation group:
                                        # ones-row (K=1) x bias-row matmul
                                        nc.tensor.matmul(
                                            ps[ot][j], ones_sb[0:1, :],
                                            scr_sb[0:1, o * 512 : (o + 1) * 512],
                                            start=False, stop=False,
                                        )
                                    nc.tensor.matmul(
                                        ps[ot][j], act_tile(k, j),
                                        w_tiles[kp][:, kk, ot, :],
                                        start=(k == 0), stop=(k == KT - 1),
                                    )
                            epilogue_half(
                                ps[ot][j], j, o,
                                nc.gpsimd if ot == 0 else nc.scalar,
                                with_bias=False,
                            )
                    else:
                        for kp in range(STAG, KT // 2):
                            for kk in range(2):
                                k = 2 * kp + kk
                                for ot in range(2):
                                    nc.tensor.matmul(
                                        ps[ot][j], act_tile(k, j),
                                        w_tiles[kp][:, kk, ot, :],
                                        start=(k == 0), stop=(k == KT - 1),
                                    )
                        epilogue_pair(
                            [ps[0][j], ps[1][j]], j, op_,
                            nc.gpsimd if j % 2 == 0 else nc.scalar,
                        )
    nc.compile()
    return nc


def _np_mmdt():
    if MM_DTYPE == "float16":
        return np.float16
    if MM_DTYPE == "bfloat16":
        import ml_dtypes

        return np.dtype(ml_dtypes.bfloat16)
    return np.float32  # float32 / float32r


def _shard_inputs(x, W, b):
    ndt = _np_mmdt()
    x = np.ascontiguousarray(x, dtype=np.float32)
    W = np.asarray(W, dtype=np.float32)
    b = np.asarray(b, dtype=np.float32)
    wQ_shards = []
    for c in range(F_SHARDS):
        wTc = np.ascontiguousarray(W[c * O_LOC : (c + 1) * O_LOC, :].T)  # [D, O_LOC]
        q = wTc.reshape(KT // 2, 2, 128, OT // 2, 2, 512)  # [kp, kt, p, pair, ot, n]
        q = q.transpose(3, 0, 2, 1, 4, 5)  # [pair, kp, p, kt, ot, n]
        wQ_shards.append(
            np.ascontiguousarray(q).astype(ndt).reshape(-1, 2 * 2 * 512)
        )
    bias_shards = [
        np.ascontiguousarray(
            np.broadcast_to(b[c * O_LOC : (c + 1) * O_LOC][None, :], (128, O_LOC))
        ).astype(np.float32)
        for c in range(F_SHARDS)
    ]
    biasr_shards = [
        b[c * O_LOC : (c + 1) * O_LOC].reshape(1, O_LOC).astype(ndt)
        for c in range(F_SHARDS)
    ]
    in_maps = []
    for core in range(M_SHARDS * F_SHARDS):
        r, c = core % M_SHARDS, core // M_SHARDS
        b_sl = slice(r * B_LOC, (r + 1) * B_LOC)
        in_maps.append(
            dict(
                xg=np.ascontiguousarray(
                    x[b_sl, :, 0].T.reshape(KG, KPG, 128, B_LOC).transpose(0, 2, 1, 3)
                ).astype(ndt).reshape(KG * 128, KPG * B_LOC),
                sN=(x[b_sl, :, 0] + x[b_sl, :, 1]).astype(ndt),
                wQ=wQ_shards[c],
                biasb=bias_shards[c],
                biasr=biasr_shards[c],
            )
        )
    return in_maps


def _gather(results):
    out = np.empty((B, D, 2), dtype=np.float32)
    val = np.empty(B, dtype=np.float32)
    for core, r in enumerate(results):
        m, c = core % M_SHARDS, core // M_SHARDS
        out[m * B_LOC : (m + 1) * B_LOC, c * O_LOC : (c + 1) * O_LOC, 0] = r["out"]
        if c == 0:
            # val_out is [128, JT] with val[j*128 + p] = arr[p, j]
            val[m * B_LOC : (m + 1) * B_LOC] = np.asarray(r["val"]).T.ravel()
    out[:, :, 1] = val[:, None]
    return out


def _run(x, W, b, trace=False, **spmd_kwargs):
    in_maps = _shard_inputs(x, W, b)
    nc = _build()
    res = run_bass_kernel_spmd(
        nc, in_maps, core_ids=list(range(8)), trace=trace, **spmd_kwargs
    )
    return _gather(res.results), res


def kernel(x, W, b):
    out, _ = _run(x, W, b, trace=False)
    return out


# revision 12
# speedup vs baseline: 1.0482x; 1.0482x over previous
"""TRN2 Bass kernel for nn_AVNNType1Linear.

Reference computation (B=2048, D_in=D_out=4096):
    act, carry = x[..., 0], x[..., 1]                  # x: [B, D_in, 2] f32
    act_out    = relu(act @ W.T + b)                   # [B, D_out]
    val        = 0.5*(mean(act, -1) + mean(carry, -1)) # [B]
    out        = stack([act_out, broadcast(val)], -1)  # [B, D_out, 2]

Distribution over 8 NeuronCores: 4-way data-parallel over batch x 2-way
tensor-parallel over output features. Per-core f16 GEMM [512b x 4096k x
2048o] = 512 matmuls of [128k x 128b] x [128k x 512o], fp32 PSUM.

The kernel is PE-issue-rate-bound: 512 MMs x ~216ns = 110.6us floor.
Key measured pitfalls this version engineers around:
  * Single-queue DMA: with every stream on the Sync HWDGE ring, w-tile
    delivery paced the matmuls to 259ns. The w stream owns the Sync ring
    exclusively; act/s/bias loads ride the Scalar (Activation) HWDGE
    ring; stores ride GpSimd SWDGE + the Scalar ring.
  * DMAHW semaphore-lane aliasing: a late s-row load (WAR-blocked behind
    the serial Scalar-engine reduction chain at sn bufs=2) shared a
    round-robin completion lane with a tail w tile and stalled the PE
    2.6us mid-kernel (then a HAM cold restart). s loads now issue early
    from a bufs=4 pool so every load completes long before the tail.
  * Startup: the PE-warmup tile is memset by the (idle) Vector engine
    instead of DMA'd, so warmup matmuls start right at preamble end and
    genuinely overlap the first w/act DMA wait (HAM at 8/8 when real
    data lands).

Traffic (~29MB/core vs 36MB before): the derive's operand arrives as a
host-prepped s = act+carry natural-layout f16 tensor (4MB) whose row
sums (-> val, the carry channel's broadcast mean) accumulate on the
otherwise-idle Scalar engine; bias leaves the PE (no ones-row K=1
matmuls) and is added in the DVE epilogue from a host-pre-broadcast
[128, O_LOC] f32 tile; the kernel stores only the act channel
([B_LOC, O_LOC] f32, 4KB DMA lines via per-(j,o-pair) [128, 1024]
tiles) plus the [B_LOC] val vector. The host assembles the interleaved
[B, D, 2] output and broadcasts val into the carry channel during
gather/unshard.

Each o-pair k-sweep keeps all 8 PSUM banks live (stationary act tile
feeds 2 consecutive matmuls) and ends with a j-major staggered tail so
every batch tile's epilogue + store streams out while later batch tiles
still compute. The very last batch tile runs its tail ot-major so the
final epilogue + store cover only [128, 512] on the fast Scalar ring.
"""

import os

import numpy as np

import concourse.mybir as mybir
import concourse.tile as tile
from concourse import bacc
from concourse.bass_utils import run_bass_kernel_spmd

B, D = 2048, 4096          # batch, D_in == D_out
M_SHARDS, F_SHARDS = 4, 2  # batch x feature grid over 8 cores
B_LOC = B // M_SHARDS      # 512 batch rows per core
O_LOC = D // F_SHARDS      # 2048 output features per core
KT = D // 128              # 32 contraction tiles
OT = O_LOC // 512          # 4 output tiles of 512
JT = B_LOC // 128          # 4 batch tiles of 128
KG = 8                     # activator SBUF tiles (groups of KT//KG k-tiles)
KPG = KT // KG

MM_DTYPE = os.environ.get("MM_DTYPE", "float16")


def _build():
    dt = mybir.dt
    mmdt = getattr(dt, MM_DTYPE)
    nc = bacc.Bacc("TRN2", target_bir_lowering=False, debug=False)
    xg = nc.dram_tensor("xg", [KG * 128, KPG * B_LOC], mmdt, kind="ExternalInput").ap()
    sN = nc.dram_tensor("sN", [B_LOC, D], mmdt, kind="ExternalInput").ap()
    wQ = nc.dram_tensor(
        "wQ", [(OT // 2) * (KT // 2) * 128, 2 * 2 * 512], mmdt, kind="ExternalInput"
    ).ap()
    biasb = nc.dram_tensor("biasb", [128, O_LOC], dt.float32, kind="ExternalInput").ap()
    biasr = nc.dram_tensor("biasr", [1, O_LOC], mmdt, kind="ExternalInput").ap()
    out = nc.dram_tensor("out", [B_LOC, O_LOC], dt.float32, kind="ExternalOutput").ap()
    val_out = nc.dram_tensor("val", [128, JT], dt.float32, kind="ExternalOutput").ap()

    with tile.TileContext(nc) as tc:
        with (
            tc.tile_pool(name="persist", bufs=1) as persist,
            tc.tile_pool(name="wstream", bufs=10) as wpool,
            tc.tile_pool(name="snpool", bufs=4) as snpool,
            tc.tile_pool(name="opool", bufs=4) as opool,
            tc.tile_pool(name="small", bufs=1) as small,
            tc.tile_pool(name="ps", bufs=8, space="PSUM") as pspool,
        ):
            # PE-warmup stationary/moving tile: engine-memset (no DMA dep)
            # so warmup matmuls start the instant the preamble ends
            ones_sb = persist.tile([128, 128], mmdt)
            nc.vector.memset(ones_sb, 1.0)
            bias_sb = persist.tile([128, O_LOC], dt.float32)

            # whole activator shard, [i%128, kt, b] layout, in KG chunks,
            # all on the Scalar HWDGE ring (keeps the Sync ring w-only).
            act_g = [
                persist.tile([128, KPG, B_LOC], mmdt, tag=f"act{g}", name=f"act{g}")
                for g in range(KG)
            ]

            def load_act(g):
                nc.scalar.dma_start(
                    out=act_g[g],
                    in_=xg[g * 128 : (g + 1) * 128, :].rearrange(
                        "p (kt b) -> p kt b", kt=KPG
                    ),
                )

            load_act(0)

            def act_tile(k, j):
                return act_g[k // KPG][:, k % KPG, j * 128 : (j + 1) * 128]

            # per-row sums of s = act+carry -> val; loads on the Scalar
            # ring, reduction on the Scalar engine (Copy accum_out). The
            # activation's main output is a don't-care: all four share one
            # scratch tile (the Scalar engine serializes them anyway).
            csum_sb = small.tile([128, JT], dt.float32)
            val_sb = small.tile([128, JT], dt.float32)
            scr_sb = small.tile([128, D], mmdt)

            def load_s(j):
                s_t = snpool.tile([128, D], mmdt, tag="sn", name=f"sn_{j}")
                nc.scalar.dma_start(out=s_t, in_=sN[j * 128 : (j + 1) * 128, :])
                return s_t

            def row_sums(s_t, j):
                nc.scalar.activation(
                    scr_sb, s_t, mybir.ActivationFunctionType.Copy,
                    accum_out=csum_sb[:, j : j + 1],
                )

            # o-pairs: all 8 PSUM banks live so each stationary act tile
            # feeds 2 consecutive matmuls (the weight-load cost amortizes)
            STAG = 13  # k-chunks before the j-major staggered tail

            def epilogue_half(ps_t, j, o, eng, with_bias=True):
                # one o-tile; 2KB-line store. with_bias=False means bias was
                # already matmul-accumulated into PSUM: single ReLU pass.
                o_sl = slice(o * 512, (o + 1) * 512)
                out_t = opool.tile(
                    [128, 512], dt.float32, tag="outh", name=f"outh_{o}_{j}"
                )
                if with_bias:
                    nc.vector.tensor_tensor(
                        out_t, ps_t, bias_sb[:, o_sl], op=mybir.AluOpType.add
                    )
                    nc.vector.tensor_scalar_max(out_t, out_t, 0.0)
                else:
                    nc.vector.tensor_scalar_max(out_t, ps_t, 0.0)
                eng.dma_start(out=out[j * 128 : (j + 1) * 128, o_sl], in_=out_t)

            def epilogue_pair(ps_pair, j, op_, eng):
                # relu(psum + bias) for both o-tiles of the pair into one
                # [128, 1024] f32 tile -> single 4KB-line store
                pair_sl = slice(2 * op_ * 512, (2 * op_ + 2) * 512)
                out_t = opool.tile(
                    [128, 2, 512], dt.float32, tag="out", name=f"out_{op_}_{j}"
                )
                for ot in range(2):
                    o_sl = slice((2 * op_ + ot) * 512, (2 * op_ + ot + 1) * 512)
                    nc.vector.tensor_tensor(
                        out_t[:, ot, :], ps_pair[ot], bias_sb[:, o_sl],
                        op=mybir.AluOpType.add,
                    )
                    nc.vector.tensor_scalar_max(out_t[:, ot, :], out_t[:, ot, :], 0.0)
                eng.dma_start(
                    out=out[j * 128 : (j + 1) * 128, pair_sl].rearrange(
                        "p (ot n) -> p ot n", n=512
                    ),
                    in_=out_t,
                )

            s_tiles = {}
            for op_ in range(OT // 2):
                o0 = 2 * op_
                pair_sl = slice(o0 * 512, (o0 + 2) * 512)
                last = op_ == OT // 2 - 1
                ps = [
                    [
                        pspool.tile(
                            [128, 512], dt.float32, tag="ps", name=f"ps_{o0}_{ot}_{j}"
                        )
                        for j in range(JT)
                    ]
                    for ot in range(2)
                ]
                if op_ == 0:
                    # PE warmup during the startup DMA wait: matmuls on the
                    # memset tile flip the HAM clock gate to 8/8 before the
                    # real data lands; the garbage lands in ps[0][0] /
                    # ps[1][0], which the first real start=True matmuls
                    # clear anyway.
                    for i in range(40):
                        nc.tensor.matmul(
                            ps[i % 2][0][:, 0:128],
                            ones_sb,
                            ones_sb,
                            start=True, stop=True,
                            skip_group_check=True,
                        )
                w_tiles = {}
                for kp in range(KT // 2):
                    if op_ == 0 and 1 <= kp <= 4:
                        load_act(2 * kp - 1)
                        if 2 * kp <= KG - 1:
                            load_act(2 * kp)
                    if op_ == 0 and kp == 6:
                        nc.scalar.dma_start(out=bias_sb, in_=biasb)
                    # the val chain (s loads -> Scalar row sums -> val store
                    # -> biasr row) runs in pair-1's DMA-quiet main loop so
                    # none of its completions can alias a DMAHW lane with a
                    # staggered-tail w tile (the measured bimodal +6us stall)
                    if op_ == 1 and 0 <= kp <= 3:
                        s_tiles[kp] = load_s(kp)
                    if op_ == 1 and 4 <= kp <= 7:
                        row_sums(s_tiles[kp - 4], kp - 4)
                    if op_ == 1 and kp == 8:
                        nc.scalar.dma_start(out=scr_sb[0:1, 0:O_LOC], in_=biasr)
                    w_t = wpool.tile(
                        [128, 2, 2, 512], mmdt, tag="wt", name=f"wt_{o0}_{kp}"
                    )
                    nc.sync.dma_start(
                        out=w_t,
                        in_=wQ[
                            (op_ * (KT // 2) + kp) * 128 : (op_ * (KT // 2) + kp + 1)
                            * 128,
                            :,
                        ].rearrange("p (kt ot n) -> p kt ot n", kt=2, ot=2),
                    )
                    w_tiles[kp] = w_t
                    if kp >= STAG:
                        continue  # tail k-chunks run j-major below
                    # two k-tiles per w fetch, two o-tiles per stationary
                    for kk in range(2):
                        k = 2 * kp + kk
                        for j in range(JT):
                            for ot in range(2):
                                nc.tensor.matmul(
                                    ps[ot][j], act_tile(k, j), w_t[:, kk, ot, :],
                                    start=(k == 0), stop=(k == KT - 1),
                                )
                if op_ == 1:
                    # val = total row sum / (2*D); tiny result, GpSimd queue
                    nc.vector.tensor_scalar_mul(val_sb, csum_sb, 1.0 / (2 * D))
                    nc.gpsimd.dma_start(out=val_out, in_=val_sb)
                # staggered j-major tail: each batch tile finishes its
                # remaining k-chunks and streams its epilogue + store out
                # while the later batch tiles still compute. The final
                # batch tile of the final pair runs ot-major so the last
                # epilogue + store cover only one [128, 512] o-tile.
                for j in range(JT):
                    final = last and j == JT - 1
                    if final:
                        for ot in range(2):
                            o = o0 + ot
                            for kp in range(STAG, KT // 2):
                                for kk in range(2):
                                    k = 2 * kp + kk
                                    if k == KT - 1:
                                        # bias rides the accumulation group:
                                        # ones-row (K=1) x bias-row matmul
                                        nc.tensor.matmul(
                                            ps[ot][j], ones_sb[0:1, :],
                                            scr_sb[0:1, o * 512 : (o + 1) * 512],
                                            start=False, stop=False,
                                        )
                                    nc.tensor.matmul(
                                        ps[ot][j], act_tile(k, j),
                                        w_tiles[kp][:, kk, ot, :],
                                        start=(k == 0), stop=(k == KT - 1),
                                    )
                            epilogue_half(
                                ps[ot][j], j, o,
                                nc.gpsimd if ot == 0 else nc.scalar,
                                with_bias=False,
                            )
                    else:
                        for kp in range(STAG, KT // 2):
                            for kk in range(2):
                                k = 2 * kp + kk
                                for ot in range(2):
                                    nc.tensor.matmul(
                                        ps[ot][j], act_tile(k, j),
                                        w_tiles[kp][:, kk, ot, :],
                                        start=(k == 0), stop=(k == KT - 1),
                                    )
                        epilogue_pair(
                            [ps[0][j], ps[1][j]], j, op_,
                            nc.gpsimd if j % 2 == 0 else nc.scalar,
                        )
    nc.compile()
    return nc


def _np_mmdt():
    if MM_DTYPE == "float16":
        return np.float16
    if MM_DTYPE == "bfloat16":
        import ml_dtypes

        return np.dtype(ml_dtypes.bfloat16)
    return np.float32  # float32 / float32r


def _shard_inputs(x, W, b):
    ndt = _np_mmdt()
    x = np.ascontiguousarray(x, dtype=np.float32)
    W = np.asarray(W, dtype=np.float32)
    b = np.asarray(b, dtype=np.float32)
    wQ_shards = []
    for c in range(F_SHARDS):
        wTc = np.ascontiguousarray(W[c * O_LOC : (c + 1) * O_LOC, :].T)  # [D, O_LOC]
        q = wTc.reshape(KT // 2, 2, 128, OT // 2, 2, 512)  # [kp, kt, p, pair, ot, n]
        q = q.transpose(3, 0, 2, 1, 4, 5)  # [pair, kp, p, kt, ot, n]
        wQ_shards.append(
            np.ascontiguousarray(q).astype(ndt).reshape(-1, 2 * 2 * 512)
        )
    bias_shards = [
        np.ascontiguousarray(
            np.broadcast_to(b[c * O_LOC : (c + 1) * O_LOC][None, :], (128, O_LOC))
        ).astype(np.float32)
        for c in range(F_SHARDS)
    ]
    biasr_shards = [
        b[c * O_LOC : (c + 1) * O_LOC].reshape(1, O_LOC).astype(ndt)
        for c in range(F_SHARDS)
    ]
    in_maps = []
    for core in range(M_SHARDS * F_SHARDS):
        r, c = core % M_SHARDS, core // M_SHARDS
        b_sl = slice(r * B_LOC, (r + 1) * B_LOC)
        in_maps.append(
            dict(
                xg=np.ascontiguousarray(
                    x[b_sl, :, 0].T.reshape(KG, KPG, 128, B_LOC).transpose(0, 2, 1, 3)
                ).astype(ndt).reshape(KG * 128, KPG * B_LOC),
                sN=(x[b_sl, :, 0] + x[b_sl, :, 1]).astype(ndt),
                wQ=wQ_shards[c],
                biasb=bias_shards[c],
                biasr=biasr_shards[c],
            )
        )
    return in_maps


def _gather(results):
    out = np.empty((B, D, 2), dtype=np.float32)
    val = np.empty(B, dtype=np.float32)
    for core, r in enumerate(results):
        m, c = core % M_SHARDS, core // M_SHARDS
        out[m * B_LOC : (m + 1) * B_LOC, c * O_LOC : (c + 1) * O_LOC, 0] = r["out"]
        if c == 0:
            # val_out is [128, JT] with val[j*128 + p] = arr[p, j]
            val[m * B_LOC : (m + 1) * B_LOC] = np.asarray(r["val"]).T.ravel()
    out[:, :, 1] = val[:, None]
    return out


def _run(x, W, b, trace=False, **spmd_kwargs):
    in_maps = _shard_inputs(x, W, b)
    nc = _build()
    res = run_bass_kernel_spmd(
        nc, in_maps, core_ids=list(range(8)), trace=trace, **spmd_kwargs
    )
    return _gather(res.results), res


def kernel(x, W, b):
    out, _ = _run(x, W, b, trace=False)
    return out
